# revision 4
# baseline (speedup 1.0000x reference)
"""Fused Conv3x3 + BatchNorm(train) + ReLU on 8 TRN2 NeuronCores.

Data-parallel over batch: each core processes 8 of the 64 images.
Conv is computed as matmuls over PSUM tiles of [128 out_ch, 512 pixels]:
the 9 filter taps are covered by 3 K=128 matmuls (kh=0,1 paired on the
partition axis) plus 3 K=64 matmuls (kh=2), accumulating in PSUM.
BatchNorm batch statistics (sum, sum-of-squares per channel) are reduced
across cores with a single small AllReduce, then scale/shift + ReLU are
applied in one scalar-engine activation pass per output chunk.
"""

import numpy as np

import concourse.bacc as bacc
import concourse.tile as tile
from concourse import mybir
from concourse.bass_utils import run_bass_kernel_spmd

N_CORES = 8
IMG_PER_CORE = 8          # 64 images / 8 cores

C_IN = 64
C_OUT = 128
H = W = 64
HP, WP = H + 2, W + 2     # zero-padded image
PIX = H * W               # 4096
TILE_PX = 512             # one PSUM bank of fp32
ROWS_PER_TILE = TILE_PX // W       # 8
TILES_PER_IMG = PIX // TILE_PX     # 8
N_TILES = IMG_PER_CORE * TILES_PER_IMG  # 64
BN_EPS = 1e-5
COUNT = 64 * H * W        # batch-stat count over (N, H, W)

F32 = mybir.dt.float32
F32R = mybir.dt.float32r
BF16 = mybir.dt.bfloat16

# Set by test harness to capture a profile; LAST_EXEC_NS holds the result.
KERNEL_TRACE = False
LAST_EXEC_NS = None

_cached_nc = None


def _round_up_pe(size):
    for v in (32, 64, 128):
        if v >= size:
            return v
    raise AssertionError(size)


def _mm_noload(nc, out, lhsT, rhs, start, stop):
    """Matmul that reuses the PE-resident weights (no LDWEIGHTS emitted).

    Mirrors bass.BassEngine.matmul but sets InstMatmult.ldweights=False;
    pair with an explicit nc.tensor.ldweights() of the same lhsT.
    """
    eng = nc.tensor
    ifmap_ap = eng.lower_ap(rhs.opt({0}), opt=False)
    weights_ap = eng.lower_ap(lhsT.opt({0}), opt=False, for_matmul_weights=True)
    out_ap = eng.lower_ap(out)
    tile_size = (_round_up_pe(rhs.partition_size()),
                 _round_up_pe(out.partition_size()))
    return eng.add_instruction(
        mybir.InstMatmult(
            name=nc.get_next_instruction_name(),
            replication_resolution=0,
            replication_shift_amnt=0,
            replication_num_rows=0,
            start_tensor_calc=start,
            stop_tensor_calc=stop,
            ins=[ifmap_ap, weights_ap],
            outs=[out_ap],
            perf_mode=None,
            is_transpose=None,
            ifmap_quant_offset=None,
            weights_quant_offset=None,
            bass_skip_group_check=True,
            tile_position=(lhsT.base_partition(), out.base_partition()),
            tile_size=tile_size,
            ldweights=False,
        )
    )


def _build():
    nc = bacc.Bacc("TRN2", target_bir_lowering=False, debug=False,
                   num_devices=N_CORES)

    x_in = nc.dram_tensor("x", [IMG_PER_CORE, 128, HP * WP], BF16,
                          kind="ExternalInput")
    wt_in = nc.dram_tensor("wt", [128, 6, 128], BF16, kind="ExternalInput")
    gb_in = nc.dram_tensor("gb", [128, 2], F32, kind="ExternalInput")
    out_d = nc.dram_tensor("out", [IMG_PER_CORE, C_OUT, PIX], F32,
                           kind="ExternalOutput")
    cc_in = nc.dram_tensor("cc_in", [128, 2], F32)
    cc_out = nc.dram_tensor("cc_out", [128, 2], F32, addr_space="Shared")

    with tile.TileContext(nc) as tc:
        with (
            tc.tile_pool(name="consts", bufs=1) as consts,
            tc.tile_pool(name="xx", bufs=2) as xx_pool,
            tc.tile_pool(name="ybuf", bufs=1) as ybuf_pool,
            tc.tile_pool(name="scratch", bufs=2) as scratch_pool,
            tc.tile_pool(name="stats", bufs=1) as stats_pool,
            tc.tile_pool(name="outp", bufs=2) as out_pool,
            tc.tile_pool(name="psum", bufs=2, space="PSUM") as psum_pool,
        ):
            wt = consts.tile([128, 6, 128], BF16)
            nc.sync.dma_start(out=wt[:], in_=wt_in[:])
            gb = consts.tile([128, 2], F32)
            nc.sync.dma_start(out=gb[:], in_=gb_in[:])
            eps_t = consts.tile([128, 1], F32)
            nc.vector.memset(eps_t[:], BN_EPS)

            # y stays resident in SBUF between the conv and the BN apply.
            ybuf = ybuf_pool.tile([128, N_TILES, TILE_PX], F32)
            sums = stats_pool.tile([128, N_TILES // 4], F32)
            sumsqs = stats_pool.tile([128, N_TILES // 4], F32)

            for img in range(IMG_PER_CORE):
                # xx: padded image, channels on partitions 0-63; partitions
                # 64-127 hold the same image shifted down one padded row so
                # (kh=0, kh=1) taps pair into one K=128 contraction.
                xx = xx_pool.tile([128, HP, WP], BF16)
                # host delivers the padded image in partitions 0-63 and the
                # one-row-shifted copy in partitions 64-127: one linear DMA
                nc.sync.dma_start(
                    out=xx[:, :, :].rearrange("p a b -> p (a b)"),
                    in_=x_in[img])

                # taps-outer over 4-tile half-images: consecutive MMs hit
                # different PSUM banks (fill/drain overlap) and each weight
                # is reused 4x per load. K=64 taps lead each bank's group so
                # K only grows within a group (shrinking K hangs the HW).
                TAPS = [(True, 0), (True, 1), (True, 2),
                        (False, 0), (False, 1), (False, 2)]
                for hf in range(2):
                    gh = img * 2 + hf
                    ps = psum_pool.tile([128, 4, TILE_PX], F32)
                    for ti, (single, kw) in enumerate(TAPS):
                        lhsT = (wt[0:64, 3 + kw, :] if single
                                else wt[:, kw, :])
                        nc.tensor.ldweights(lhsT, tile_position=(0, 0))
                        for tp in range(4):
                            h0 = (hf * 4 + tp) * ROWS_PER_TILE
                            if single:
                                rhs = xx[0:64,
                                         h0 + 2:h0 + 2 + ROWS_PER_TILE,
                                         kw:kw + W]
                            else:
                                rhs = xx[:, h0:h0 + ROWS_PER_TILE,
                                         kw:kw + W]
                            _mm_noload(
                                nc, ps[:, tp, :], lhsT, rhs,
                                start=(ti == 0), stop=(ti == 5),
                            )
                    gt4 = img * TILES_PER_IMG + hf * 4
                    # PSUM -> SBUF copy + per-channel sum over all 4 banks
                    nc.scalar.activation(
                        ybuf[:, gt4:gt4 + 4, :], ps[:],
                        mybir.ActivationFunctionType.Copy,
                        accum_out=sums[:, gh:gh + 1],
                    )
                    # square + per-channel sum of squares (on DVE)
                    sc = scratch_pool.tile([128, 4, TILE_PX], F32)
                    nc.vector.tensor_mul(sc[:], ybuf[:, gt4:gt4 + 4, :],
                                         ybuf[:, gt4:gt4 + 4, :])
                    nc.vector.reduce_sum(sumsqs[:, gh:gh + 1], sc[:],
                                         axis=mybir.AxisListType.XY)

            # fold per-tile partials, all-reduce across the 8 cores
            st = stats_pool.tile([128, 2], F32)
            nc.vector.reduce_sum(st[:, 0:1], sums[:],
                                 axis=mybir.AxisListType.X)
            nc.vector.reduce_sum(st[:, 1:2], sumsqs[:],
                                 axis=mybir.AxisListType.X)
            nc.sync.dma_start(out=cc_in[:], in_=st[:])
            nc.gpsimd.collective_compute(
                "AllReduce",
                mybir.AluOpType.add,
                ins=[cc_in[:]],
                outs=[cc_out[:]],
                replica_groups=[list(range(N_CORES))],
            )
            g = stats_pool.tile([128, 2], F32)
            nc.sync.dma_start(out=g[:], in_=cc_out[:])

            # scale = gamma * rsqrt(var + eps); shift = beta - scale * mean
            mean = stats_pool.tile([128, 1], F32)
            m2 = stats_pool.tile([128, 1], F32)
            var = stats_pool.tile([128, 1], F32)
            sd = stats_pool.tile([128, 1], F32)
            inv = stats_pool.tile([128, 1], F32)
            scl = stats_pool.tile([128, 1], F32)
            shv = stats_pool.tile([128, 1], F32)
            tmp = stats_pool.tile([128, 1], F32)
            nc.vector.tensor_scalar_mul(mean[:], g[:, 0:1], 1.0 / COUNT)
            nc.vector.tensor_scalar_mul(m2[:], g[:, 1:2], 1.0 / COUNT)
            nc.vector.tensor_mul(tmp[:], mean[:], mean[:])
            nc.vector.tensor_sub(var[:], m2[:], tmp[:])
            nc.scalar.activation(sd[:], var[:],
                                 mybir.ActivationFunctionType.Sqrt,
                                 bias=eps_t[:])
            nc.vector.reciprocal(inv[:], sd[:])
            nc.vector.tensor_mul(scl[:], gb[:, 0:1], inv[:])
            nc.vector.tensor_mul(tmp[:], scl[:], mean[:])
            nc.vector.tensor_sub(shv[:], gb[:, 1:2], tmp[:])

            # apply: out = relu(y * scale + shift), in half-image chunks
            CH_TILES = 4  # tiles per chunk
            for img in range(IMG_PER_CORE):
                for half in range(TILES_PER_IMG // CH_TILES):
                    t0 = img * TILES_PER_IMG + half * CH_TILES
                    ot = out_pool.tile([128, CH_TILES, TILE_PX], F32)
                    nc.scalar.activation(
                        ot[:], ybuf[:, t0:t0 + CH_TILES, :],
                        mybir.ActivationFunctionType.Relu,
                        bias=shv[:], scale=scl[:],
                    )
                    px0 = half * CH_TILES * TILE_PX
                    nc.sync.dma_start(
                        out=out_d[img, :, px0:px0 + CH_TILES * TILE_PX],
                        in_=ot[:],
                    )

    nc.compile()
    return nc


def _prep_weights(weight: np.ndarray) -> np.ndarray:
    # [p, q, mb, mb] block matrix -> truncated OIHW kernel [128, 64, 3, 3]
    p, q, mb, _ = weight.shape
    Wm = weight.transpose(0, 2, 1, 3).reshape(p * mb, q * mb)
    Wm = Wm[:C_OUT, :C_IN * 9].reshape(C_OUT, C_IN, 3, 3)
    wt = np.zeros((128, 6, 128), np.float32)
    # pairs: partition c -> (kh=0), partition 64+c -> (kh=1)
    wt[:64, 0:3, :] = Wm[:, :, 0, :].transpose(1, 2, 0)
    wt[64:, 0:3, :] = Wm[:, :, 1, :].transpose(1, 2, 0)
    # singles (kh=2), duplicated in both partition halves
    wt[:64, 3:6, :] = Wm[:, :, 2, :].transpose(1, 2, 0)
    wt[64:, 3:6, :] = Wm[:, :, 2, :].transpose(1, 2, 0)
    import ml_dtypes
    return wt.astype(np.dtype(ml_dtypes.bfloat16))


def kernel(x, weight, gamma, beta):
    global _cached_nc, LAST_EXEC_NS
    x = np.asarray(x, np.float32)
    weight = np.asarray(weight, np.float32)
    gamma = np.asarray(gamma, np.float32)
    beta = np.asarray(beta, np.float32)

    if _cached_nc is None:
        _cached_nc = _build()
    nc = _cached_nc

    wt = _prep_weights(weight)
    gb = np.ascontiguousarray(np.stack([gamma, beta], axis=1))
    import ml_dtypes
    bf16 = np.dtype(ml_dtypes.bfloat16)
    xp = np.zeros((64, 128, HP * WP), bf16)
    pad = np.zeros((64, C_IN, HP, WP), np.float32)
    pad[:, :, 1:H + 1, 1:W + 1] = x
    pad = pad.reshape(64, C_IN, HP * WP).astype(bf16)
    xp[:, :C_IN, :] = pad
    xp[:, C_IN:, :HP * WP - WP] = pad[:, :, WP:]
    in_maps = []
    for i in range(N_CORES):
        shard = np.ascontiguousarray(
            xp[i * IMG_PER_CORE:(i + 1) * IMG_PER_CORE])
        in_maps.append({"x": shard, "wt": wt, "gb": gb})

    res = run_bass_kernel_spmd(nc, in_maps, list(range(N_CORES)),
                               trace=KERNEL_TRACE)
    LAST_EXEC_NS = res.exec_time_ns

    out = np.concatenate(
        [res.results[i]["out"].reshape(IMG_PER_CORE, C_OUT, H, W)
         for i in range(N_CORES)], axis=0)
    return out



# revision 8
# speedup vs baseline: 1.1497x; 1.1497x over previous
"""Fused Conv3x3 + BatchNorm(train) + ReLU on 8 TRN2 NeuronCores.

Data-parallel over batch: each core processes 8 of the 64 images.
Conv is computed as matmuls over PSUM tiles of [128 out_ch, 512 pixels].
The 9 filter taps are covered by 5 matmuls per tile: 4 K=128 pairs plus
one K=64 single. Pairing uses two input layouts per image:
  xa: partitions 0-63 padded image, 64-127 same image shifted down one
      padded row  -> pairs (kh=0,kw)+(kh=1,kw)
  xb: partitions 0-63 padded image, 64-127 same image shifted left one
      element     -> pair  (kh=2,0)+(kh=2,1)
leaving (kh=2,kw=2) as the lone K=64 tap (issued first so K only grows
within a PSUM accumulation group).

Weight loads are amortized: each tap's LDWEIGHTS is followed by one
matmul per PSUM bank; redundant LDWEIGHTS that the tile legalizer
emits per-matmul are deleted post-compile (_dedupe_ldweights).

BatchNorm batch statistics: per-channel sum rides the PSUM->SBUF copy
(scalar activation accum), sum-of-squares is one DVE
tensor_tensor_reduce straight from PSUM. Per-device partials are
AllReduced (1KB), then scale/shift + ReLU are applied in chunked
scalar-engine activation passes overlapping the output DMA.
"""

import numpy as np

import concourse.bacc as bacc
import concourse.tile as tile
from concourse import mybir
from concourse.bass_utils import run_bass_kernel_spmd

N_CORES = 8
IMG_PER_CORE = 8          # 64 images / 8 cores
C_IN = 64
C_OUT = 128
H = W = 64
HP, WP = H + 2, W + 2     # zero-padded image
PIX = H * W               # 4096
TILE_PX = 512             # one PSUM bank of fp32
ROWS_PER_TILE = TILE_PX // W       # 8
TILES_PER_IMG = PIX // TILE_PX     # 8
BN_EPS = 1e-5
COUNT = 64 * H * W        # batch-stat count over (N, H, W)

F32 = mybir.dt.float32
BF16 = mybir.dt.bfloat16

# Set by test harness to capture a profile; LAST_EXEC_NS holds the result.
KERNEL_TRACE = False
LAST_EXEC_NS = None

_cached_nc = None

# conv groups: (img, first_tile, n_tiles). Last image uses small groups
# so the post-conv stats tail (copy + sumsq of the final group) is short.
GROUPS = [(i, t, 4) for i in range(7) for t in (0, 4)]
GROUPS += [(7, t, 2) for t in (0, 2, 4, 6)]
NG = len(GROUPS)          # 18

# apply-phase chunks: (first_tile_global, n_tiles); small leading chunks
# prime the output-DMA pipe sooner.
CHUNKS = [(0, 1), (1, 1), (2, 2), (4, 4)]
CHUNKS += [(8 * i + t, 4) for i in range(1, 8) for t in (0, 4)]


def _dedupe_ldweights(nc):
    """Delete redundant InstLdweights the legalizer emits per-matmul.

    Consecutive matmuls that reuse the PE-resident weights keep only the
    first load. Only sync-free duplicates whose key (tensor, offset,
    pattern, dtype, PE tiling) matches the previous load are removed.
    """
    removed = 0
    for f in nc.m.functions:
        for blk in f.blocks:
            insts = blk.instructions
            keep = []
            last_key = None
            for i in insts:
                tn = type(i).__name__
                if tn == 'InstLdweights':
                    a = i.ins[0]
                    key = (a.memref, a.offset, str(a.ap), str(a.dtype),
                           i.tile_position, i.tile_size,
                           str(i.perf_mode), i.is_transpose)
                    si = i.sync_info
                    clean = si is None or (not si.on_wait and not si.on_update)
                    if clean and key == last_key:
                        removed += 1
                        continue
                    last_key = key
                elif tn != 'InstMatmult':
                    last_key = None
                keep.append(i)
            if removed and len(keep) != len(insts):
                del insts[:]
                for i in keep:
                    insts.append(i)
    return removed


def _build():
    nc = bacc.Bacc("TRN2", target_bir_lowering=False, debug=False,
                   num_devices=N_CORES)

    xa_in = nc.dram_tensor("xa", [IMG_PER_CORE, 128, HP * WP], BF16,
                           kind="ExternalInput")
    xb_in = nc.dram_tensor("xb", [IMG_PER_CORE, 128, HP * WP], BF16,
                           kind="ExternalInput")
    wt_in = nc.dram_tensor("wt", [128, 5, 128], BF16, kind="ExternalInput")
    gb_in = nc.dram_tensor("gb", [128, 2], F32, kind="ExternalInput")
    out_d = nc.dram_tensor("out", [IMG_PER_CORE, C_OUT, PIX], F32,
                           kind="ExternalOutput")
    cc_in = nc.dram_tensor("cc_in", [128, 2], F32)
    cc_out = nc.dram_tensor("cc_out", [128, 2], F32, addr_space="Shared")

    with tile.TileContext(nc) as tc:
        with (
            tc.tile_pool(name="consts", bufs=1) as consts,
            tc.tile_pool(name="xa", bufs=2) as xa_pool,
            tc.tile_pool(name="xb", bufs=2) as xb_pool,
            tc.tile_pool(name="ybuf", bufs=1) as ybuf_pool,
            tc.tile_pool(name="sq", bufs=2) as sq_pool,
            tc.tile_pool(name="stats", bufs=1) as stats_pool,
            tc.tile_pool(name="outp", bufs=4) as out_pool,
            tc.tile_pool(name="psum", bufs=2, space="PSUM") as psum_pool,
        ):
            wt = consts.tile([128, 5, 128], BF16)
            nc.sync.dma_start(out=wt[:], in_=wt_in[:])
            gb = consts.tile([128, 2], F32)
            nc.sync.dma_start(out=gb[:], in_=gb_in[:])
            eps_t = consts.tile([128, 1], F32)
            nc.vector.memset(eps_t[:], BN_EPS)

            # y stays resident in SBUF (bf16) between conv and BN apply.
            ybuf = ybuf_pool.tile([128, 64, TILE_PX], BF16)
            sums = stats_pool.tile([128, NG], F32)
            sumsqs = stats_pool.tile([128, NG], F32)

            xa_t = {}
            xb_t = {}

            def load_image(img, split):
                xa = xa_pool.tile([128, HP, WP], BF16)
                xb = xb_pool.tile([128, HP, WP], BF16)
                if split:
                    # land the rows the first tile group reads first
                    nc.sync.dma_start(
                        out=xa[:, 0:36, :].rearrange("p a b -> p (a b)"),
                        in_=xa_in[img, :, 0:36 * WP])
                    nc.sync.dma_start(
                        out=xb[:, 0:36, :].rearrange("p a b -> p (a b)"),
                        in_=xb_in[img, :, 0:36 * WP])
                    nc.sync.dma_start(
                        out=xa[:, 36:, :].rearrange("p a b -> p (a b)"),
                        in_=xa_in[img, :, 36 * WP:])
                    nc.sync.dma_start(
                        out=xb[:, 36:, :].rearrange("p a b -> p (a b)"),
                        in_=xb_in[img, :, 36 * WP:])
                else:
                    nc.sync.dma_start(
                        out=xa[:, :, :].rearrange("p a b -> p (a b)"),
                        in_=xa_in[img])
                    nc.sync.dma_start(
                        out=xb[:, :, :].rearrange("p a b -> p (a b)"),
                        in_=xb_in[img])
                xa_t[img] = xa
                xb_t[img] = xb

            load_image(0, split=True)

            for g, (img, t0, nt) in enumerate(GROUPS):
                if t0 == 0 and img + 1 < IMG_PER_CORE:
                    load_image(img + 1, split=False)
                xa, xb = xa_t[img], xb_t[img]
                ps = psum_pool.tile([128, nt, TILE_PX], F32)
                for ti in range(5):
                    for tp in range(nt):
                        h0 = (t0 + tp) * ROWS_PER_TILE
                        if ti == 0:        # single tap (kh=2, kw=2), K=64
                            lhsT = wt[0:64, 0, :]
                            rhs = xa[0:64, h0 + 2:h0 + 10, 2:2 + W]
                        elif ti < 4:       # pairs (0,kw)+(1,kw), K=128
                            kw = ti - 1
                            lhsT = wt[:, ti, :]
                            rhs = xa[:, h0:h0 + 8, kw:kw + W]
                        else:              # pair (2,0)+(2,1), K=128
                            lhsT = wt[:, 4, :]
                            rhs = xb[:, h0 + 2:h0 + 10, 0:W]
                        nc.tensor.matmul(
                            ps[:, tp, :], lhsT=lhsT, rhs=rhs,
                            start=(ti == 0), stop=(ti == 4),
                            skip_group_check=True,
                        )
                gt = img * TILES_PER_IMG + t0
                # PSUM -> SBUF copy + per-channel sum (scalar engine)
                nc.scalar.activation(
                    ybuf[:, gt:gt + nt, :], ps[:],
                    mybir.ActivationFunctionType.Copy,
                    accum_out=sums[:, g:g + 1],
                )
                # per-channel sum of squares in one DVE pass:
                # sq = (y * 1) * y, accum_out = sum(sq)
                yb = ybuf[:, gt:gt + nt, :]
                sq = sq_pool.tile([128, nt, TILE_PX], BF16)
                nc.vector.scalar_tensor_tensor(
                    out=sq[:], in0=yb, scalar=1.0, in1=yb,
                    op0=mybir.AluOpType.mult, op1=mybir.AluOpType.mult,
                    accum_out=sumsqs[:, g:g + 1],
                )

            # fold per-group partials, all-reduce across the 8 cores
            st = stats_pool.tile([128, 2], F32)
            nc.vector.reduce_sum(st[:, 0:1], sums[:],
                                 axis=mybir.AxisListType.X)
            nc.vector.reduce_sum(st[:, 1:2], sumsqs[:],
                                 axis=mybir.AxisListType.X)
            nc.sync.dma_start(out=cc_in[:], in_=st[:])
            nc.gpsimd.collective_compute(
                "AllReduce",
                mybir.AluOpType.add,
                ins=[cc_in[:]],
                outs=[cc_out[:]],
                replica_groups=[list(range(N_CORES))],
            )
            g = stats_pool.tile([128, 2], F32)
            nc.sync.dma_start(out=g[:], in_=cc_out[:])

            # scale = gamma * rsqrt(var + eps); shift = beta - scale * mean
            mv = stats_pool.tile([128, 2], F32)   # [mean, E[y^2]]
            var = stats_pool.tile([128, 1], F32)
            sd = stats_pool.tile([128, 1], F32)
            inv = stats_pool.tile([128, 1], F32)
            scl = stats_pool.tile([128, 1], F32)
            shv = stats_pool.tile([128, 1], F32)
            tmp = stats_pool.tile([128, 1], F32)
            nc.vector.tensor_scalar_mul(mv[:], g[:], 1.0 / COUNT)
            nc.vector.tensor_mul(tmp[:], mv[:, 0:1], mv[:, 0:1])
            nc.vector.tensor_sub(var[:], mv[:, 1:2], tmp[:])
            nc.scalar.activation(sd[:], var[:],
                                 mybir.ActivationFunctionType.Sqrt,
                                 bias=eps_t[:])
            nc.vector.reciprocal(inv[:], sd[:])
            nc.vector.tensor_mul(scl[:], gb[:, 0:1], inv[:])
            nc.vector.tensor_mul(tmp[:], scl[:], mv[:, 0:1])
            nc.vector.tensor_sub(shv[:], gb[:, 1:2], tmp[:])

            # apply: out = relu(y * scale + shift), overlapping output DMA
            for t0, nt in CHUNKS:
                img, tl = divmod(t0, TILES_PER_IMG)
                ot = out_pool.tile([128, nt, TILE_PX], F32)
                nc.scalar.activation(
                    ot[:], ybuf[:, t0:t0 + nt, :],
                    mybir.ActivationFunctionType.Relu,
                    bias=shv[:], scale=scl[:],
                )
                px0 = tl * TILE_PX
                nc.sync.dma_start(
                    out=out_d[img, :, px0:px0 + nt * TILE_PX],
                    in_=ot[:],
                )

    nc.compile()
    import os
    if os.environ.get("NO_DEDUPE") != "1":
        _dedupe_ldweights(nc)
    return nc


def _prep_weights(weight: np.ndarray) -> np.ndarray:
    # [p, q, mb, mb] block matrix -> truncated OIHW kernel [128, 64, 3, 3]
    p, q, mb, _ = weight.shape
    Wm = weight.transpose(0, 2, 1, 3).reshape(p * mb, q * mb)
    Wm = Wm[:C_OUT, :C_IN * 9].reshape(C_OUT, C_IN, 3, 3)
    wt = np.zeros((128, 5, 128), np.float32)
    wt[:64, 0, :] = Wm[:, :, 2, 2].T          # single (2,2), K=64
    for k in range(3):                        # pairs (0,k)+(1,k)
        wt[:64, 1 + k, :] = Wm[:, :, 0, k].T
        wt[64:, 1 + k, :] = Wm[:, :, 1, k].T
    wt[:64, 4, :] = Wm[:, :, 2, 0].T          # pair (2,0)+(2,1)
    wt[64:, 4, :] = Wm[:, :, 2, 1].T
    import ml_dtypes
    return wt.astype(np.dtype(ml_dtypes.bfloat16))


def kernel(x, weight, gamma, beta):
    global _cached_nc, LAST_EXEC_NS
    x = np.asarray(x, np.float32)
    weight = np.asarray(weight, np.float32)
    gamma = np.asarray(gamma, np.float32)
    beta = np.asarray(beta, np.float32)

    if _cached_nc is None:
        _cached_nc = _build()
    nc = _cached_nc

    wt = _prep_weights(weight)
    gb = np.ascontiguousarray(np.stack([gamma, beta], axis=1))
    import ml_dtypes
    bf16 = np.dtype(ml_dtypes.bfloat16)
    pad = np.zeros((64, C_IN, HP, WP), np.float32)
    pad[:, :, 1:H + 1, 1:W + 1] = x
    flat = pad.reshape(64, C_IN, HP * WP).astype(bf16)
    xa = np.zeros((64, 128, HP * WP), bf16)
    xa[:, :C_IN, :] = flat
    xa[:, C_IN:, :HP * WP - WP] = flat[:, :, WP:]   # shift down one row
    xb = np.zeros((64, 128, HP * WP), bf16)
    xb[:, :C_IN, :] = flat
    xb[:, C_IN:, :HP * WP - 1] = flat[:, :, 1:]     # shift left one elem
    in_maps = []
    for i in range(N_CORES):
        sl = slice(i * IMG_PER_CORE, (i + 1) * IMG_PER_CORE)
        in_maps.append({
            "xa": np.ascontiguousarray(xa[sl]),
            "xb": np.ascontiguousarray(xb[sl]),
            "wt": wt, "gb": gb,
        })

    res = run_bass_kernel_spmd(nc, in_maps, list(range(N_CORES)),
                               trace=KERNEL_TRACE)
    LAST_EXEC_NS = res.exec_time_ns

    out = np.concatenate(
        [res.results[i]["out"].reshape(IMG_PER_CORE, C_OUT, H, W)
         for i in range(N_CORES)], axis=0)
    return out


# revision 9
# speedup vs baseline: 1.2053x; 1.0483x over previous
"""Fused Conv3x3 + BatchNorm(train) + ReLU on 8 TRN2 NeuronCores.

Data-parallel over batch: each core processes 8 of the 64 images.
Conv is computed as matmuls over PSUM tiles of [128 out_ch, 512 pixels].
The 9 filter taps are covered by 5 matmuls per tile: 4 K=128 pairs plus
one K=64 single. Pairing uses two input layouts per image:
  xa: partitions 0-63 padded image, 64-127 same image shifted down one
      padded row  -> pairs (kh=0,kw)+(kh=1,kw)
  xb: partitions 0-63 padded image, 64-127 same image shifted left one
      element     -> pair  (kh=2,0)+(kh=2,1)
leaving (kh=2,kw=2) as the lone K=64 tap (issued first so K only grows
within a PSUM accumulation group).

Weight loads are amortized: each tap's LDWEIGHTS is followed by one
matmul per PSUM bank; redundant LDWEIGHTS that the tile legalizer
emits per-matmul are deleted post-compile (_dedupe_ldweights).

BatchNorm batch statistics: per-channel sum rides the PSUM->SBUF copy
(scalar activation accum), sum-of-squares is one DVE
tensor_tensor_reduce straight from PSUM. Per-device partials are
AllReduced (1KB), then scale/shift + ReLU are applied in chunked
scalar-engine activation passes overlapping the output DMA.
"""

import numpy as np

import concourse.bacc as bacc
import concourse.tile as tile
from concourse import mybir
from concourse.bass_utils import run_bass_kernel_spmd

N_CORES = 8
IMG_PER_CORE = 8          # 64 images / 8 cores
C_IN = 64
C_OUT = 128
H = W = 64
HP, WP = H + 2, W + 2     # zero-padded image
PIX = H * W               # 4096
TILE_PX = 512             # one PSUM bank of fp32
ROWS_PER_TILE = TILE_PX // W       # 8
TILES_PER_IMG = PIX // TILE_PX     # 8
BN_EPS = 1e-5
COUNT = 64 * H * W        # batch-stat count over (N, H, W)

F32 = mybir.dt.float32
BF16 = mybir.dt.bfloat16

# Set by test harness to capture a profile; LAST_EXEC_NS holds the result.
KERNEL_TRACE = False
LAST_EXEC_NS = None

_cached_nc = None

# conv groups: (img, first_tile, n_tiles). Last image uses small groups
# so the post-conv stats tail (copy + sumsq of the final group) is short.
GROUPS = [(i, t, 4) for i in range(7) for t in (0, 4)]
GROUPS += [(7, t, 2) for t in (0, 2, 4, 6)]
NG = len(GROUPS)          # 18

# apply-phase chunks: (first_tile_global, n_tiles); small leading chunks
# prime the output-DMA pipe sooner.
CHUNKS = [(0, 1), (1, 1), (2, 2), (4, 4)]
CHUNKS += [(8 * i + t, 4) for i in range(1, 8) for t in (0, 4)]


def _dedupe_ldweights(nc):
    """Delete redundant InstLdweights the legalizer emits per-matmul.

    Consecutive matmuls that reuse the PE-resident weights keep only the
    first load. Only sync-free duplicates whose key (tensor, offset,
    pattern, dtype, PE tiling) matches the previous load are removed.
    """
    removed = 0
    for f in nc.m.functions:
        for blk in f.blocks:
            insts = blk.instructions
            keep = []
            last_key = None
            for i in insts:
                tn = type(i).__name__
                if tn == 'InstLdweights':
                    a = i.ins[0]
                    key = (a.memref, a.offset, str(a.ap), str(a.dtype),
                           i.tile_position, i.tile_size,
                           str(i.perf_mode), i.is_transpose)
                    si = i.sync_info
                    clean = si is None or (not si.on_wait and not si.on_update)
                    if clean and key == last_key:
                        removed += 1
                        continue
                    last_key = key
                elif tn != 'InstMatmult':
                    last_key = None
                keep.append(i)
            if removed and len(keep) != len(insts):
                del insts[:]
                for i in keep:
                    insts.append(i)
    return removed


def _build():
    nc = bacc.Bacc("TRN2", target_bir_lowering=False, debug=False,
                   num_devices=N_CORES)

    # Clear kernel-range semaphores at entry. The target_bir_lowering=False
    # path skips the per-kernel sem_clear, so stale semaphore values left by
    # a previous (crashed or foreign) kernel on the shared device would
    # satisfy this kernel's >=N waits early and corrupt results.
    from concourse.bass import compact_to_ranges
    for sem_range in compact_to_ranges(
            [s for s in nc._kernel_sem_range if s not in nc.barrier_sems]):
        nc.gpsimd.dma_reset(sem_range)
        nc.gpsimd.sem_clear(sem_range)
    nc._nrt_pseudo_barrier()

    xa_in = nc.dram_tensor("xa", [IMG_PER_CORE, 128, HP * WP], BF16,
                           kind="ExternalInput")
    xb_in = nc.dram_tensor("xb", [IMG_PER_CORE, 128, HP * WP], BF16,
                           kind="ExternalInput")
    wt_in = nc.dram_tensor("wt", [128, 5, 128], BF16, kind="ExternalInput")
    gb_in = nc.dram_tensor("gb", [128, 2], F32, kind="ExternalInput")
    out_d = nc.dram_tensor("out", [IMG_PER_CORE, C_OUT, PIX], F32,
                           kind="ExternalOutput")
    cc_in = nc.dram_tensor("cc_in", [128, 2], F32)
    cc_out = nc.dram_tensor("cc_out", [128, 2], F32, addr_space="Shared")

    with tile.TileContext(nc) as tc:
        with (
            tc.tile_pool(name="consts", bufs=1) as consts,
            tc.tile_pool(name="xa", bufs=2) as xa_pool,
            tc.tile_pool(name="xb", bufs=2) as xb_pool,
            tc.tile_pool(name="ybuf", bufs=1) as ybuf_pool,
            tc.tile_pool(name="sq", bufs=2) as sq_pool,
            tc.tile_pool(name="stats", bufs=1) as stats_pool,
            tc.tile_pool(name="outp", bufs=4) as out_pool,
            tc.tile_pool(name="psum", bufs=2, space="PSUM") as psum_pool,
        ):
            wt = consts.tile([128, 5, 128], BF16)
            nc.sync.dma_start(out=wt[:], in_=wt_in[:])
            gb = consts.tile([128, 2], F32)
            nc.sync.dma_start(out=gb[:], in_=gb_in[:])
            eps_t = consts.tile([128, 1], F32)
            nc.vector.memset(eps_t[:], BN_EPS)

            # y stays resident in SBUF (bf16) between conv and BN apply.
            ybuf = ybuf_pool.tile([128, 64, TILE_PX], BF16)
            sums = stats_pool.tile([128, NG], F32)
            sumsqs = stats_pool.tile([128, NG], F32)

            xa_t = {}
            xb_t = {}

            def load_image(img, split):
                xa = xa_pool.tile([128, HP, WP], BF16)
                xb = xb_pool.tile([128, HP, WP], BF16)
                if split:
                    # land the rows the first tile group reads first
                    nc.sync.dma_start(
                        out=xa[:, 0:36, :].rearrange("p a b -> p (a b)"),
                        in_=xa_in[img, :, 0:36 * WP])
                    nc.sync.dma_start(
                        out=xb[:, 0:36, :].rearrange("p a b -> p (a b)"),
                        in_=xb_in[img, :, 0:36 * WP])
                    nc.sync.dma_start(
                        out=xa[:, 36:, :].rearrange("p a b -> p (a b)"),
                        in_=xa_in[img, :, 36 * WP:])
                    nc.sync.dma_start(
                        out=xb[:, 36:, :].rearrange("p a b -> p (a b)"),
                        in_=xb_in[img, :, 36 * WP:])
                else:
                    nc.sync.dma_start(
                        out=xa[:, :, :].rearrange("p a b -> p (a b)"),
                        in_=xa_in[img])
                    nc.sync.dma_start(
                        out=xb[:, :, :].rearrange("p a b -> p (a b)"),
                        in_=xb_in[img])
                xa_t[img] = xa
                xb_t[img] = xb

            load_image(0, split=True)

            for g, (img, t0, nt) in enumerate(GROUPS):
                if t0 == 0 and img + 1 < IMG_PER_CORE:
                    load_image(img + 1, split=False)
                xa, xb = xa_t[img], xb_t[img]
                ps = psum_pool.tile([128, nt, TILE_PX], F32)
                for ti in range(5):
                    for tp in range(nt):
                        h0 = (t0 + tp) * ROWS_PER_TILE
                        if ti == 0:        # single tap (kh=2, kw=2), K=64
                            lhsT = wt[0:64, 0, :]
                            rhs = xa[0:64, h0 + 2:h0 + 10, 2:2 + W]
                        elif ti < 4:       # pairs (0,kw)+(1,kw), K=128
                            kw = ti - 1
                            lhsT = wt[:, ti, :]
                            rhs = xa[:, h0:h0 + 8, kw:kw + W]
                        else:              # pair (2,0)+(2,1), K=128
                            lhsT = wt[:, 4, :]
                            rhs = xb[:, h0 + 2:h0 + 10, 0:W]
                        nc.tensor.matmul(
                            ps[:, tp, :], lhsT=lhsT, rhs=rhs,
                            start=(ti == 0), stop=(ti == 4),
                            skip_group_check=True,
                        )
                gt = img * TILES_PER_IMG + t0
                # PSUM -> SBUF copy + per-channel sum (scalar engine)
                nc.scalar.activation(
                    ybuf[:, gt:gt + nt, :], ps[:],
                    mybir.ActivationFunctionType.Copy,
                    accum_out=sums[:, g:g + 1],
                )
                # per-channel sum of squares in one DVE pass:
                # sq = (y * 1) * y, accum_out = sum(sq)
                yb = ybuf[:, gt:gt + nt, :]
                sq = sq_pool.tile([128, nt, TILE_PX], BF16)
                nc.vector.scalar_tensor_tensor(
                    out=sq[:], in0=yb, scalar=1.0, in1=yb,
                    op0=mybir.AluOpType.mult, op1=mybir.AluOpType.mult,
                    accum_out=sumsqs[:, g:g + 1],
                )

            # fold per-group partials, all-reduce across the 8 cores
            st = stats_pool.tile([128, 2], F32)
            nc.vector.reduce_sum(st[:, 0:1], sums[:],
                                 axis=mybir.AxisListType.X)
            nc.vector.reduce_sum(st[:, 1:2], sumsqs[:],
                                 axis=mybir.AxisListType.X)
            nc.sync.dma_start(out=cc_in[:], in_=st[:])
            nc.gpsimd.collective_compute(
                "AllReduce",
                mybir.AluOpType.add,
                ins=[cc_in[:]],
                outs=[cc_out[:]],
                replica_groups=[list(range(N_CORES))],
            )
            g = stats_pool.tile([128, 2], F32)
            nc.sync.dma_start(out=g[:], in_=cc_out[:])

            # scale = gamma * rsqrt(var + eps); shift = beta - scale * mean
            mv = stats_pool.tile([128, 2], F32)   # [mean, E[y^2]]
            var = stats_pool.tile([128, 1], F32)
            sd = stats_pool.tile([128, 1], F32)
            inv = stats_pool.tile([128, 1], F32)
            scl = stats_pool.tile([128, 1], F32)
            shv = stats_pool.tile([128, 1], F32)
            tmp = stats_pool.tile([128, 1], F32)
            nc.vector.tensor_scalar_mul(mv[:], g[:], 1.0 / COUNT)
            nc.vector.tensor_mul(tmp[:], mv[:, 0:1], mv[:, 0:1])
            nc.vector.tensor_sub(var[:], mv[:, 1:2], tmp[:])
            nc.scalar.activation(sd[:], var[:],
                                 mybir.ActivationFunctionType.Sqrt,
                                 bias=eps_t[:])
            nc.vector.reciprocal(inv[:], sd[:])
            nc.vector.tensor_mul(scl[:], gb[:, 0:1], inv[:])
            nc.vector.tensor_mul(tmp[:], scl[:], mv[:, 0:1])
            nc.vector.tensor_sub(shv[:], gb[:, 1:2], tmp[:])

            # apply: out = relu(y * scale + shift), overlapping output DMA
            for t0, nt in CHUNKS:
                img, tl = divmod(t0, TILES_PER_IMG)
                ot = out_pool.tile([128, nt, TILE_PX], F32)
                nc.scalar.activation(
                    ot[:], ybuf[:, t0:t0 + nt, :],
                    mybir.ActivationFunctionType.Relu,
                    bias=shv[:], scale=scl[:],
                )
                px0 = tl * TILE_PX
                nc.sync.dma_start(
                    out=out_d[img, :, px0:px0 + nt * TILE_PX],
                    in_=ot[:],
                )

    nc.compile()
    import os
    if os.environ.get("NO_DEDUPE") != "1":
        _dedupe_ldweights(nc)
    return nc


def _prep_weights(weight: np.ndarray) -> np.ndarray:
    # [p, q, mb, mb] block matrix -> truncated OIHW kernel [128, 64, 3, 3]
    p, q, mb, _ = weight.shape
    Wm = weight.transpose(0, 2, 1, 3).reshape(p * mb, q * mb)
    Wm = Wm[:C_OUT, :C_IN * 9].reshape(C_OUT, C_IN, 3, 3)
    wt = np.zeros((128, 5, 128), np.float32)
    wt[:64, 0, :] = Wm[:, :, 2, 2].T          # single (2,2), K=64
    for k in range(3):                        # pairs (0,k)+(1,k)
        wt[:64, 1 + k, :] = Wm[:, :, 0, k].T
        wt[64:, 1 + k, :] = Wm[:, :, 1, k].T
    wt[:64, 4, :] = Wm[:, :, 2, 0].T          # pair (2,0)+(2,1)
    wt[64:, 4, :] = Wm[:, :, 2, 1].T
    import ml_dtypes
    return wt.astype(np.dtype(ml_dtypes.bfloat16))


def kernel(x, weight, gamma, beta):
    global _cached_nc, LAST_EXEC_NS
    x = np.asarray(x, np.float32)
    weight = np.asarray(weight, np.float32)
    gamma = np.asarray(gamma, np.float32)
    beta = np.asarray(beta, np.float32)

    if _cached_nc is None:
        _cached_nc = _build()
    nc = _cached_nc

    wt = _prep_weights(weight)
    gb = np.ascontiguousarray(np.stack([gamma, beta], axis=1))
    import ml_dtypes
    bf16 = np.dtype(ml_dtypes.bfloat16)
    pad = np.zeros((64, C_IN, HP, WP), np.float32)
    pad[:, :, 1:H + 1, 1:W + 1] = x
    flat = pad.reshape(64, C_IN, HP * WP).astype(bf16)
    xa = np.zeros((64, 128, HP * WP), bf16)
    xa[:, :C_IN, :] = flat
    xa[:, C_IN:, :HP * WP - WP] = flat[:, :, WP:]   # shift down one row
    xb = np.zeros((64, 128, HP * WP), bf16)
    xb[:, :C_IN, :] = flat
    xb[:, C_IN:, :HP * WP - 1] = flat[:, :, 1:]     # shift left one elem
    in_maps = []
    for i in range(N_CORES):
        sl = slice(i * IMG_PER_CORE, (i + 1) * IMG_PER_CORE)
        in_maps.append({
            "xa": np.ascontiguousarray(xa[sl]),
            "xb": np.ascontiguousarray(xb[sl]),
            "wt": wt, "gb": gb,
        })

    res = run_bass_kernel_spmd(nc, in_maps, list(range(N_CORES)),
                               trace=KERNEL_TRACE)
    LAST_EXEC_NS = res.exec_time_ns

    out = np.concatenate(
        [res.results[i]["out"].reshape(IMG_PER_CORE, C_OUT, H, W)
         for i in range(N_CORES)], axis=0)
    return out
